# revision 10
# baseline (speedup 1.0000x reference)
"""Trainium2 Bass kernel for YOLO-style detection decode (nms_detection).

Computes, for input `output` (B=8, H=80, W=80, A*85=255):
  per (b, cell, anchor):  xy = (sigmoid(txy) + grid_off) * stride
                          wh = exp(twh) * anchor
                          bbox = [xy - wh/2, xy + wh/2]
                          p_c = sigmoid(cls_c) * sigmoid(obj)
  out (B, C*hw*A, 6) rows = [cid, score, x1, y1, x2, y2] where
  cid = c if p_c > 0.01 else -1, score = p_c if p_c > 0.01 else 0.

Sharding: pure data parallel over batch, one batch element per NeuronCore.

Per-core layout strategy (output is 37 MB/core -> write-bandwidth bound):
  - cells are processed in supertiles (ramped sizes: small first/last tiles to
    shorten pipeline fill/drain); within a supertile of ns*128 cells, cell
    c0 + p*ns + s lives on partition p, subtile s, so the input DMA reads
    contiguous ns*1020B per partition (large descriptors).
  - class scores are transposed (TensorE, f32 to keep thresholding exact) to
    class-major (80 partitions) so the per-class output block (hw, A, 6) is
    DMA'd with contiguous per-partition segments
  - bbox (class-independent) is broadcast to all 80 class partitions with
    selector matmuls (K=24, bf16) whose rhs is a block-checkerboard (even
    columns carry pair-value x, odd columns y), so PSUM comes out
    cell-interleaved [u, 2] and the PSUM->SBUF scatter writes 8B segments
    instead of 4B-strided singles. bbox is computed in f32 then rounded to
    bf16 (bounded 2^-9 relative error; no thresholds involved).
  - exp(x) is computed as sigmoid(x)/sigmoid(-x) so the ScalarE activation
    table never leaves the sigmoid set (a table switch costs ~2.7us).
"""

import sys
import os
from contextlib import ExitStack

if "/opt/trn_rl_repo" not in sys.path:
    sys.path.insert(0, "/opt/trn_rl_repo")

import numpy as np

NUM_CLASSES = 80
NUM_ANCHOR = 3
NUM_PRED = 85
HW_CELLS = 6400
THRESH = 0.01
N_CORES = 8
ROW = 6 * NUM_ANCHOR  # f32 per cell per class in the output (18)

_CACHE = {}
LAST_RESULT = None  # BassKernelResults of the most recent kernel() call

RAMP = os.environ.get("KERNEL_RAMP", "1") == "1"
QBF16 = os.environ.get("KERNEL_QBF16", "0") == "1"  # bf16 matmul PSUM output (unsupported on TRN2)
# how many of the 6 bbox copies go on DVE (rest on ACT)
NVEC_COPY = int(os.environ.get("KERNEL_NVEC_COPY", "2"))


def _st_sizes():
    if RAMP:
        sizes = [128, 256] + [512] * 11 + [256, 128]
    else:
        sizes = [512] * 12 + [256]
    assert sum(sizes) == HW_CELLS
    return sizes


def _build(stride_f: float):
    import concourse.bass as bass  # noqa: F401
    import concourse.bacc as bacc
    import concourse.tile as tile
    from concourse import mybir

    f32 = mybir.dt.float32
    bf16 = mybir.dt.bfloat16
    AF = mybir.ActivationFunctionType
    OP = mybir.AluOpType

    C = NUM_CLASSES
    A = NUM_ANCHOR

    st_sizes = _st_sizes()
    n_off_cols = sum(6 * (n // 128) for n in st_sizes)
    CONST_F = n_off_cols + 24 + 128 + 1 + 12 * C  # offs | hanch | ident | cp1 | sel
    OFF_HANCH = n_off_cols
    OFF_IDENT = OFF_HANCH + 24
    OFF_CP1 = OFF_IDENT + 128
    OFF_SEL = OFF_CP1 + 1

    q_dt = bf16 if QBF16 else f32

    nc = bacc.Bacc("TRN2", target_bir_lowering=False, debug=False)
    x_d = nc.declare_dram_parameter("x", [HW_CELLS, A * NUM_PRED], f32, isOutput=False)
    const_d = nc.declare_dram_parameter("consts", [128, CONST_F], f32, isOutput=False)
    out_d = nc.declare_dram_parameter("out", [C, HW_CELLS * ROW], f32, isOutput=True)

    with ExitStack() as ctx:
        tc = ctx.enter_context(tile.TileContext(nc))
        cpool = ctx.enter_context(tc.tile_pool(name="const", bufs=1))
        in_pool = ctx.enter_context(tc.tile_pool(name="inp", bufs=3))
        sig_pool = ctx.enter_context(tc.tile_pool(name="sig", bufs=2))
        sm_pool = ctx.enter_context(tc.tile_pool(name="small", bufs=2))
        s_pool = ctx.enter_context(tc.tile_pool(name="scls", bufs=2))
        m_pool = ctx.enter_context(tc.tile_pool(name="mask", bufs=3))
        stg_pool = ctx.enter_context(tc.tile_pool(name="stage", bufs=2))
        o_pool = ctx.enter_context(tc.tile_pool(name="outt", bufs=3))
        p_pool = ctx.enter_context(tc.tile_pool(name="ppsum", bufs=2, space="PSUM"))
        q_pool = ctx.enter_context(
            tc.tile_pool(name="qpsum", bufs=(6 if QBF16 else 3), space="PSUM")
        )

        # ---- constants (one DMA -> one sem lane) ----
        const_sb = cpool.tile([128, CONST_F], f32, tag="consts")
        nc.scalar.dma_start(out=const_sb[:, :], in_=const_d[:, :])
        offs_sb = const_sb[:, 0:OFF_HANCH]
        hanch_sb = const_sb[:, OFF_HANCH:OFF_IDENT]
        ident_sb = const_sb[:, OFF_IDENT:OFF_CP1]
        cp1_sb = const_sb[:, OFF_CP1:OFF_SEL]
        sel_sb = const_sb[:, OFF_SEL:CONST_F]
        sel_r = cpool.tile([128, 12 * C], bf16, tag="selr")
        nc.vector.tensor_copy(sel_r[:, :], sel_sb)

        # ---- warm-up: let each engine observe the const DMA once, so no
        # later instruction needs more than one sync-wait (ISA limit) ----
        warm = cpool.tile([128, 4], f32, tag="warm")
        nc.vector.tensor_copy(warm[0:1, 0:1], const_sb[0:1, 0:1])
        nc.scalar.copy(warm[0:1, 1:2], const_sb[0:1, 0:1])
        nc.gpsimd.tensor_copy(warm[0:1, 2:3], const_sb[0:1, 0:1])
        wq = p_pool.tile([128, 128], f32, tag="P")
        nc.tensor.transpose(wq[:, :], ident_sb, ident_sb)

        c0 = 0
        off_col = 0
        for st, ncell in enumerate(st_sizes):
            ns = ncell // 128  # subtiles (cells per partition)

            # ---- load input supertile: partition p holds cells
            # c0 + p*ns .. c0 + p*ns + ns-1 (contiguous ns*1020B descriptors)
            in_t = in_pool.tile([128, ns * 255], f32, tag="in")
            nc.scalar.dma_start(
                out=in_t[:, :].rearrange("p (s c) -> p s c", c=255),
                in_=x_d[c0 : c0 + ncell, :].rearrange("(p s) c -> p s c", s=ns),
            )

            # ---- cell-major transforms ----
            sig = sig_pool.tile([128, ns * 255], f32, tag="sig")
            nc.scalar.activation(sig[:, :], in_t[:, :], AF.Sigmoid)

            in_v = in_t[:, :].rearrange("p (s a c) -> p s a c", a=A, c=NUM_PRED)
            sig_v = sig[:, :].rearrange("p (s a c) -> p s a c", a=A, c=NUM_PRED)

            # exp(wh) = sigmoid(wh) / sigmoid(-wh)
            sgnw = sm_pool.tile([128, ns * 6], f32, tag="sgnw")
            nc.scalar.activation(
                sgnw[:, :].rearrange("p (s a k) -> p s a k", a=A, k=2),
                in_v[:, :, :, 2:4],
                AF.Sigmoid,
                scale=-1.0,
            )
            rec = sm_pool.tile([128, ns * 6], f32, tag="rec")
            nc.vector.reciprocal(rec[:, :], sgnw[:, :])
            t1 = sm_pool.tile([128, ns * 6], f32, tag="t1")
            nc.vector.tensor_tensor(
                t1[:, :].rearrange("p (s a k) -> p s a k", a=A, k=2),
                sig_v[:, :, :, 2:4],
                hanch_sb[:, : ns * 6].rearrange("p (s a k) -> p s a k", a=A, k=2),
                OP.mult,
            )
            halfwh = sm_pool.tile([128, ns * 6], f32, tag="halfwh")
            nc.vector.tensor_tensor(halfwh[:, :], t1[:, :], rec[:, :], OP.mult)

            # xy = sigmoid(xy)*stride + off*stride
            xy = sm_pool.tile([128, ns * 6], f32, tag="xy")
            nc.vector.scalar_tensor_tensor(
                xy[:, :].rearrange("p (s a k) -> p s a k", a=A, k=2),
                in0=sig_v[:, :, :, 0:2],
                scalar=stride_f,
                in1=offs_sb[:, off_col : off_col + ns * 6].rearrange(
                    "p (s a k) -> p s a k", a=A, k=2
                ),
                op0=OP.mult,
                op1=OP.add,
            )

            # per-subtile block layout [S_a0 | S_a1 | S_a2 | pad 16 | bb 12]:
            # anchor 2's transpose carries the bbox columns, landing them on
            # partitions 96..107
            SW = A * C + 16 + 12  # 268
            S = s_pool.tile([128, ns * SW], f32, tag="S")

            S_v = S[:, :].rearrange("p (s w) -> p s w", w=SW)
            bb_v = S_v[:, :, A * C + 16 : A * C + 28].rearrange(
                "p s (a k) -> p s a k", k=4
            )
            xy_v = xy[:, :].rearrange("p (s a k) -> p s a k", a=A, k=2)
            hw_v = halfwh[:, :].rearrange("p (s a k) -> p s a k", a=A, k=2)
            nc.vector.tensor_tensor(bb_v[:, :, :, 0:2], xy_v, hw_v, OP.subtract)
            nc.vector.tensor_tensor(bb_v[:, :, :, 2:4], xy_v, hw_v, OP.add)

            # class scores = sigmoid(cls) * sigmoid(obj), cell-major; obj is
            # broadcast along the class dim with a stride-0 AP
            nc.gpsimd.tensor_tensor(
                S_v[:, :, 0 : A * C].rearrange("p s (a c) -> p s a c", c=C),
                sig_v[:, :, :, 5:85],
                sig_v[:, :, :, 4:5].to_broadcast([128, ns, A, C]),
                OP.mult,
            )

            # output supertile, class-major; cell index i = p*ns + s
            outt = o_pool.tile([C, ncell * ROW], f32, tag="outt")
            # ov_e[c, e, s, p] = outt[c, (p*ns+s)*18 + e]
            ov_e = outt[:, :].rearrange("c (p s e) -> c e s p", s=ns, e=ROW)
            # ov_pair[c, s, p, e]: for 8B-segment pair writes
            ov_pair = outt[:, :].rearrange("c (p s e) -> c s p e", s=ns, e=ROW)

            bbt = stg_pool.tile([128, ncell], bf16, tag="bbt")

            ncopy = 0
            for a in (2, 0, 1):  # anchor 2 first: it stages the bbox rows
                # transpose scores of anchor a -> [C, (s,p)]; anchor 2 also
                # carries the 12 bbox rows into partitions 96..107
                pw = C + 28 if a == 2 else C
                P = p_pool.tile([C + 28, ncell], f32, tag="P")
                for s in range(ns):
                    nc.tensor.transpose(
                        P[0:pw, s * 128 : (s + 1) * 128],
                        S_v[:, s, a * C : a * C + pw],
                        ident_sb[:, :],
                    )
                if a == 2:
                    # stage bbox rows to SBUF (cast to bf16)
                    nc.vector.tensor_copy(bbt[96:108, :], P[96:108, :])
                P_v = P[0:C, :].rearrange("c (s p) -> c s p", p=128)
                mask = m_pool.tile([C, ncell], bf16, tag="mask")
                nc.vector.tensor_scalar(mask[:, :], P[0:C, :], THRESH, None, OP.is_gt)
                mask_v = mask[:, :].rearrange("c (s p) -> c s p", p=128)
                # score -> column a*6+1 (strided 18)
                nc.vector.tensor_tensor(
                    ov_e[:, a * 6 + 1, :, :], P_v, mask_v, OP.mult
                )
                # cid = mask*(c+1) - 1 -> column a*6+0 (gpsimd, SBUF-only op)
                nc.gpsimd.tensor_scalar(
                    ov_e[:, a * 6 + 0, :, :],
                    mask_v,
                    cp1_sb[0:C, :],
                    -1.0,
                    OP.mult,
                    OP.add,
                )

                # bbox broadcast: per value-pair, two K=12 matmuls write the
                # even/odd slots of a cell-interleaved PSUM tile [c, u, 2]
                # (out cols at 8B stride); rhs is the plain bbt both times.
                for pr in range(2):
                    q = q_pool.tile([C, 2 * ncell], f32, tag="q")
                    q_v = q[:, :].rearrange("c (h u m) -> c h u m", h=2, m=2)
                    for h in range(2):
                        hc = ncell // 2
                        for m in range(2):
                            j = a * 4 + 2 * pr + m
                            nc.tensor.matmul(
                                q_v[:, h, :, m],
                                lhsT=sel_r[96:108, j * C : (j + 1) * C],
                                rhs=bbt[96:108, h * hc : (h + 1) * hc],
                                start=True,
                                stop=True,
                                tile_position=(96, 0),
                            )
                    dst = ov_pair[:, :, :, a * 6 + 2 + 2 * pr : a * 6 + 4 + 2 * pr]
                    src = q[:, :].rearrange("c (s p m) -> c s p m", p=128, m=2)
                    if ncopy < NVEC_COPY:
                        nc.vector.tensor_copy(dst, src)
                    else:
                        nc.scalar.copy(dst, src)
                    ncopy += 1

            # ---- store ----
            nc.sync.dma_start(
                out=out_d[:, c0 * ROW : (c0 + ncell) * ROW], in_=outt[:, :]
            )
            c0 += ncell
            off_col += ns * 6

    nc.finalize()
    return nc


def make_consts(anchor, offset, stride_f):
    """Pack [offs | hanch | ident | cp1 | sel] into one (128, F) f32 blob."""
    st_sizes = _st_sizes()
    off = np.asarray(offset, dtype=np.float32).reshape(-1, 2)[:HW_CELLS] * stride_f
    blocks = []
    c0 = 0
    for ncell in st_sizes:
        ns = ncell // 128
        blk = off[c0 : c0 + ncell].reshape(128, ns, 1, 2)  # [p, s, 1, k]
        blk = np.broadcast_to(blk, (128, ns, 3, 2)).reshape(128, ns * 6)
        blocks.append(blk)
        c0 += ncell
    offs_cols = np.ascontiguousarray(np.concatenate(blocks, axis=1))
    a2 = np.asarray(anchor, dtype=np.float32).reshape(NUM_ANCHOR, 2)
    hanch = np.tile((a2 / 2.0).reshape(6), (128, 4)).astype(np.float32)
    ident = np.eye(128, dtype=np.float32)
    cp1 = np.broadcast_to(np.arange(1, 129, dtype=np.float32).reshape(128, 1), (128, 1))
    # one-hot selector for bbox channel j on PE rows 96..107
    sel128 = np.zeros((128, 12 * NUM_CLASSES), dtype=np.float32)
    for j in range(12):
        sel128[96 + j, j * NUM_CLASSES : (j + 1) * NUM_CLASSES] = 1.0
    blob = np.concatenate([offs_cols, hanch, ident, cp1, sel128], axis=1)
    return np.ascontiguousarray(blob.astype(np.float32))


def _host_prep(output, anchor, offset, stride):
    stride_f = float(stride)
    B = output.shape[0]
    x_all = np.ascontiguousarray(
        np.asarray(output, dtype=np.float32).reshape(B, HW_CELLS, NUM_ANCHOR * NUM_PRED)
    )
    consts = make_consts(anchor, offset, stride_f)
    return stride_f, x_all, consts


def kernel(output, anchor, offset, stride):
    from concourse.bass_utils import run_bass_kernel_spmd

    stride_f, x_all, consts = _host_prep(output, anchor, offset, stride)
    key = ("nc", stride_f, QBF16, RAMP, NVEC_COPY)
    if key not in _CACHE:
        _CACHE[key] = _build(stride_f)
    nc = _CACHE[key]

    in_maps = [{"x": x_all[b], "consts": consts} for b in range(N_CORES)]
    res = run_bass_kernel_spmd(
        nc,
        in_maps,
        list(range(N_CORES)),
        tmpdir=os.environ.get("KERNEL_TRACE_DIR") or None,
    )
    global LAST_RESULT
    LAST_RESULT = res
    outs = [
        r["out"].reshape(NUM_CLASSES * HW_CELLS * NUM_ANCHOR, 6) for r in res.results
    ]
    return np.stack(outs, axis=0)


if __name__ == "__main__":
    rng = np.random.default_rng(0)
    out = rng.standard_normal((8, 80, 80, 255), dtype=np.float32)
    anchor = rng.uniform(10.0, 120.0, (1, 1, 3, 2)).astype(np.float32)
    gy, gx = np.meshgrid(np.arange(80, dtype=np.float32), np.arange(80, dtype=np.float32), indexing="ij")
    offset = np.stack([gx, gy], axis=-1).reshape(1, 80, 80, 1, 2)
    r = kernel(out, anchor, offset, 8)
    print(r.shape, r.dtype)


# revision 29
# speedup vs baseline: 1.0672x; 1.0672x over previous
"""Trainium2 Bass kernel for YOLO-style detection decode (nms_detection).

Computes, for input `output` (B=8, H=80, W=80, A*85=255):
  per (b, cell, anchor):  xy = (sigmoid(txy) + grid_off) * stride
                          wh = exp(twh) * anchor
                          bbox = [xy - wh/2, xy + wh/2]
                          p_c = sigmoid(cls_c) * sigmoid(obj)
  out (B, C*hw*A, 6) rows = [cid, score, x1, y1, x2, y2] where
  cid = c if p_c > 0.01 else -1, score = p_c if p_c > 0.01 else 0.

Sharding: pure data parallel over batch, one batch element per NeuronCore.

Per-core layout strategy (output is 37 MB/core -> write-bandwidth bound):
  - cells are processed in supertiles (ramped sizes: small first/last tiles to
    shorten pipeline fill/drain); within a supertile of ns*128 cells, cell
    c0 + p*ns + s lives on partition p, subtile s, so the input DMA reads
    contiguous ns*1020B per partition (large descriptors).
  - class scores are transposed (TensorE, f32 to keep thresholding exact) to
    class-major (80 partitions) so the per-class output block (hw, A, 6) is
    DMA'd with contiguous per-partition segments
  - bbox (class-independent) is broadcast to all 80 class partitions with
    selector matmuls (K=24, bf16) whose rhs is a block-checkerboard (even
    columns carry pair-value x, odd columns y), so PSUM comes out
    cell-interleaved [u, 2] and the PSUM->SBUF scatter writes 8B segments
    instead of 4B-strided singles. bbox is computed in f32 then rounded to
    bf16 (bounded 2^-9 relative error; no thresholds involved).
  - exp(x) is computed as sigmoid(x)/sigmoid(-x) so the ScalarE activation
    table never leaves the sigmoid set (a table switch costs ~2.7us).
"""

import sys
import os
from contextlib import ExitStack

if "/opt/trn_rl_repo" not in sys.path:
    sys.path.insert(0, "/opt/trn_rl_repo")

import numpy as np

NUM_CLASSES = 80
NUM_ANCHOR = 3
NUM_PRED = 85
HW_CELLS = 6400
THRESH = 0.01
N_CORES = 8
ROW = 6 * NUM_ANCHOR  # f32 per cell per class in the output (18)

_CACHE = {}
LAST_RESULT = None  # BassKernelResults of the most recent kernel() call

RAMP = os.environ.get("KERNEL_RAMP", "1") == "1"
QBF16 = os.environ.get("KERNEL_QBF16", "0") == "1"  # bf16 matmul PSUM output (unsupported on TRN2)
# how many of the 6 bbox copies go on DVE (rest on ACT)
NVEC_COPY = int(os.environ.get("KERNEL_NVEC_COPY", "1"))
IN_BUFS = int(os.environ.get("KERNEL_IN_BUFS", "4"))
O_BUFS = int(os.environ.get("KERNEL_O_BUFS", "4"))


def _st_sizes():
    if RAMP:
        sizes = [128, 256] + [512] * 11 + [256, 128]
    else:
        sizes = [512] * 12 + [256]
    assert sum(sizes) == HW_CELLS
    return sizes


def _build(stride_f: float):
    import concourse.bass as bass  # noqa: F401
    import concourse.bacc as bacc
    import concourse.tile as tile
    from concourse import mybir

    f32 = mybir.dt.float32
    bf16 = mybir.dt.bfloat16
    AF = mybir.ActivationFunctionType
    OP = mybir.AluOpType

    C = NUM_CLASSES
    A = NUM_ANCHOR

    st_sizes = _st_sizes()
    n_off_cols = sum(6 * (n // 128) for n in st_sizes)
    CONST_F = n_off_cols + 24 + 128 + 1 + 6 * C  # offs | hanch | ident | cp1 | sel
    OFF_HANCH = n_off_cols
    OFF_IDENT = OFF_HANCH + 24
    OFF_CP1 = OFF_IDENT + 128
    OFF_SEL = OFF_CP1 + 1

    q_dt = bf16 if QBF16 else f32

    nc = bacc.Bacc("TRN2", target_bir_lowering=False, debug=False)
    x_d = nc.declare_dram_parameter("x", [HW_CELLS, A * NUM_PRED], f32, isOutput=False)
    const_d = nc.declare_dram_parameter("consts", [128, CONST_F], f32, isOutput=False)
    out_d = nc.declare_dram_parameter("out", [C, HW_CELLS * ROW], f32, isOutput=True)

    with ExitStack() as ctx:
        tc = ctx.enter_context(tile.TileContext(nc))
        cpool = ctx.enter_context(tc.tile_pool(name="const", bufs=1))
        in_pool = ctx.enter_context(tc.tile_pool(name="inp", bufs=IN_BUFS))
        sig_pool = ctx.enter_context(tc.tile_pool(name="sig", bufs=2))
        sm_pool = ctx.enter_context(tc.tile_pool(name="small", bufs=2))
        s_pool = ctx.enter_context(tc.tile_pool(name="scls", bufs=2))
        m_pool = ctx.enter_context(tc.tile_pool(name="mask", bufs=3))
        o_pool = ctx.enter_context(tc.tile_pool(name="outt", bufs=O_BUFS))
        p_pool = ctx.enter_context(tc.tile_pool(name="ppsum", bufs=2, space="PSUM"))
        p2_pool = ctx.enter_context(tc.tile_pool(name="p2psum", bufs=2, space="PSUM"))
        q_pool = ctx.enter_context(tc.tile_pool(name="qpsum", bufs=2, space="PSUM"))

        # ---- constants (one DMA -> one sem lane) ----
        const_sb = cpool.tile([128, CONST_F], f32, tag="consts")
        nc.scalar.dma_start(out=const_sb[:, :], in_=const_d[:, :])
        offs_sb = const_sb[:, 0:OFF_HANCH]
        hanch_sb = const_sb[:, OFF_HANCH:OFF_IDENT]
        ident_sb = const_sb[:, OFF_IDENT:OFF_CP1]
        cp1_sb = const_sb[:, OFF_CP1:OFF_SEL]
        sel_sb = const_sb[:, OFF_SEL:CONST_F]
        sel_r = cpool.tile([128, 6 * C], bf16, tag="selr")
        nc.vector.tensor_copy(sel_r[:, :], sel_sb)

        # persistent checkerboard rhs buffers (double-buffered by supertile):
        # rows 96..107 carry bbox values at even columns, rows 0..11 at odd
        # columns, rows 12..95 stay zero forever (memset once here).
        MAXQ = 2 * 512
        bbt2_a = cpool.tile([108, MAXQ], bf16, tag="bbt2_a")
        bbt2_b = cpool.tile([108, MAXQ], bf16, tag="bbt2_b")
        bbt2_bufs = [bbt2_a, bbt2_b]
        for b in bbt2_bufs:
            nc.vector.memset(b[:, :], 0.0)

        # ---- warm-up: let each engine observe the const DMA once, so no
        # later instruction needs more than one sync-wait (ISA limit) ----
        warm = cpool.tile([128, 4], f32, tag="warm")
        nc.vector.tensor_copy(warm[0:1, 0:1], const_sb[0:1, 0:1])
        nc.scalar.copy(warm[0:1, 1:2], const_sb[0:1, 0:1])
        nc.gpsimd.tensor_copy(warm[0:1, 2:3], const_sb[0:1, 0:1])
        wq = p_pool.tile([128, 128], f32, tag="P")
        nc.tensor.transpose(wq[:, :], ident_sb, ident_sb)
        # ~5us of back-to-back PE work: flips the HAM clock gate to 8/8
        # (2.4 GHz) before the pipeline starts; later inter-burst gaps are
        # shorter than the re-throttle window so the PE stays warm.
        for _ in range(28):
            nc.tensor.transpose(wq[:, :], ident_sb, ident_sb)

        c0 = 0
        off_col = 0
        for st, ncell in enumerate(st_sizes):
            ns = ncell // 128  # subtiles (cells per partition)

            # ---- load input supertile: partition p holds cells
            # c0 + p*ns .. c0 + p*ns + ns-1 (contiguous ns*1020B descriptors)
            in_t = in_pool.tile([128, ns * 255], f32, tag="in")
            nc.scalar.dma_start(
                out=in_t[:, :].rearrange("p (s c) -> p s c", c=255),
                in_=x_d[c0 : c0 + ncell, :].rearrange("(p s) c -> p s c", s=ns),
            )

            # ---- cell-major transforms ----
            sig = sig_pool.tile([128, ns * 255], f32, tag="sig")
            nc.scalar.activation(sig[:, :], in_t[:, :], AF.Sigmoid)

            in_v = in_t[:, :].rearrange("p (s a c) -> p s a c", a=A, c=NUM_PRED)
            sig_v = sig[:, :].rearrange("p (s a c) -> p s a c", a=A, c=NUM_PRED)

            # exp(wh) = sigmoid(wh) / sigmoid(-wh)
            sgnw = sm_pool.tile([128, ns * 6], f32, tag="sgnw")
            nc.scalar.activation(
                sgnw[:, :].rearrange("p (s a k) -> p s a k", a=A, k=2),
                in_v[:, :, :, 2:4],
                AF.Sigmoid,
                scale=-1.0,
            )
            rec = sm_pool.tile([128, ns * 6], f32, tag="rec")
            nc.vector.reciprocal(rec[:, :], sgnw[:, :])
            t1 = sm_pool.tile([128, ns * 6], f32, tag="t1")
            nc.vector.tensor_tensor(
                t1[:, :].rearrange("p (s a k) -> p s a k", a=A, k=2),
                sig_v[:, :, :, 2:4],
                hanch_sb[:, : ns * 6].rearrange("p (s a k) -> p s a k", a=A, k=2),
                OP.mult,
            )
            halfwh = sm_pool.tile([128, ns * 6], f32, tag="halfwh")
            nc.vector.tensor_tensor(halfwh[:, :], t1[:, :], rec[:, :], OP.mult)

            # xy = sigmoid(xy)*stride + off*stride
            xy = sm_pool.tile([128, ns * 6], f32, tag="xy")
            nc.vector.scalar_tensor_tensor(
                xy[:, :].rearrange("p (s a k) -> p s a k", a=A, k=2),
                in0=sig_v[:, :, :, 0:2],
                scalar=stride_f,
                in1=offs_sb[:, off_col : off_col + ns * 6].rearrange(
                    "p (s a k) -> p s a k", a=A, k=2
                ),
                op0=OP.mult,
                op1=OP.add,
            )

            # per-subtile block layout [S_a0 | S_a1 | S_a2 | pad 16 | bb 12]:
            # anchor 2's transpose carries the bbox columns, landing them on
            # partitions 96..107
            SW = A * C + 16 + 12  # 268
            S = s_pool.tile([128, ns * SW], f32, tag="S")

            S_v = S[:, :].rearrange("p (s w) -> p s w", w=SW)
            bb_v = S_v[:, :, A * C + 16 : A * C + 28].rearrange(
                "p s (a k) -> p s a k", k=4
            )
            xy_v = xy[:, :].rearrange("p (s a k) -> p s a k", a=A, k=2)
            hw_v = halfwh[:, :].rearrange("p (s a k) -> p s a k", a=A, k=2)
            nc.vector.tensor_tensor(bb_v[:, :, :, 0:2], xy_v, hw_v, OP.subtract)
            nc.vector.tensor_tensor(bb_v[:, :, :, 2:4], xy_v, hw_v, OP.add)

            # class scores = sigmoid(cls) * sigmoid(obj), cell-major; obj is
            # broadcast along the class dim with a stride-0 AP
            nc.gpsimd.tensor_tensor(
                S_v[:, :, 0 : A * C].rearrange("p s (a c) -> p s a c", c=C),
                sig_v[:, :, :, 5:85],
                sig_v[:, :, :, 4:5].to_broadcast([128, ns, A, C]),
                OP.mult,
            )

            # output supertile, class-major; cell index i = p*ns + s
            outt = o_pool.tile([C, ncell * ROW], f32, tag="outt")
            # ov_e[c, e, s, p] = outt[c, (p*ns+s)*18 + e]
            ov_e = outt[:, :].rearrange("c (p s e) -> c e s p", s=ns, e=ROW)
            # ov_pair[c, s, p, e]: for 8B-segment pair writes
            ov_pair = outt[:, :].rearrange("c (p s e) -> c s p e", s=ns, e=ROW)

            bbt2 = bbt2_bufs[st % 2]
            bbt2_v = bbt2[:, 0 : 2 * ncell].rearrange("r (u m) -> r u m", m=2)

            # tiny transposes: bbox rows to partitions 0..11 (for odd slots)
            P2 = p2_pool.tile([12, ncell], f32, tag="P2")
            for s in range(ns):
                nc.tensor.transpose(
                    P2[0:12, s * 128 : (s + 1) * 128],
                    S_v[:, s, A * C + 16 : A * C + 28],
                    ident_sb[:, :],
                )
            nc.scalar.copy(bbt2_v[0:12, :, 1], P2[0:12, :])

            ncopy = 0
            for a in (2, 0, 1):  # anchor 2 first: it stages the bbox rows
                # transpose scores of anchor a -> [C, (s,p)]; anchor 2 also
                # carries the 12 bbox rows into partitions 96..107
                pw = C + 28 if a == 2 else C
                P = p_pool.tile([C + 28, ncell], f32, tag="P")
                for s in range(ns):
                    nc.tensor.transpose(
                        P[0:pw, s * 128 : (s + 1) * 128],
                        S_v[:, s, a * C : a * C + pw],
                        ident_sb[:, :],
                    )
                if a == 2:
                    # stage bbox rows into the even slots (cast to bf16)
                    nc.scalar.copy(bbt2_v[96:108, :, 0], P[96:108, :])
                P_v = P[0:C, :].rearrange("c (s p) -> c s p", p=128)
                mask = m_pool.tile([C, ncell], bf16, tag="mask")
                nc.vector.tensor_scalar(mask[:, :], P[0:C, :], THRESH, None, OP.is_gt)
                mask_v = mask[:, :].rearrange("c (s p) -> c s p", p=128)
                # score -> column a*6+1 (strided 18)
                nc.vector.tensor_tensor(
                    ov_e[:, a * 6 + 1, :, :], P_v, mask_v, OP.mult
                )
                # cid = mask*(c+1) - 1 -> column a*6+0 (gpsimd, SBUF-only op)
                nc.gpsimd.tensor_scalar(
                    ov_e[:, a * 6 + 0, :, :],
                    mask_v,
                    cp1_sb[0:C, :],
                    -1.0,
                    OP.mult,
                    OP.add,
                )

                # bbox broadcast: per value-pair one K=108 selector matmul per
                # half, reading the checkerboard rhs -> contiguous PSUM
                # [c, u, 2] (cell-interleaved pairs, no stride penalty)
                for pr in range(2):
                    blk = (a * 2 + pr) * C
                    q = q_pool.tile([C, 2 * ncell], f32, tag="q")
                    for h in range(2):
                        nc.tensor.matmul(
                            q[:, h * ncell : (h + 1) * ncell],
                            lhsT=sel_r[0:108, blk : blk + C],
                            rhs=bbt2[0:108, h * ncell : (h + 1) * ncell],
                            start=True,
                            stop=True,
                        )
                    dst = ov_pair[:, :, :, a * 6 + 2 + 2 * pr : a * 6 + 4 + 2 * pr]
                    src = q[:, :].rearrange("c (s p m) -> c s p m", p=128, m=2)
                    if ncopy < NVEC_COPY:
                        nc.vector.tensor_copy(dst, src)
                    else:
                        nc.scalar.copy(dst, src)
                    ncopy += 1

            # ---- store ----
            nc.sync.dma_start(
                out=out_d[:, c0 * ROW : (c0 + ncell) * ROW], in_=outt[:, :]
            )
            c0 += ncell
            off_col += ns * 6

    nc.finalize()
    return nc


def make_consts(anchor, offset, stride_f):
    """Pack [offs | hanch | ident | cp1 | sel] into one (128, F) f32 blob."""
    st_sizes = _st_sizes()
    off = np.asarray(offset, dtype=np.float32).reshape(-1, 2)[:HW_CELLS] * stride_f
    blocks = []
    c0 = 0
    for ncell in st_sizes:
        ns = ncell // 128
        blk = off[c0 : c0 + ncell].reshape(128, ns, 1, 2)  # [p, s, 1, k]
        blk = np.broadcast_to(blk, (128, ns, 3, 2)).reshape(128, ns * 6)
        blocks.append(blk)
        c0 += ncell
    offs_cols = np.ascontiguousarray(np.concatenate(blocks, axis=1))
    a2 = np.asarray(anchor, dtype=np.float32).reshape(NUM_ANCHOR, 2)
    hanch = np.tile((a2 / 2.0).reshape(6), (128, 4)).astype(np.float32)
    ident = np.eye(128, dtype=np.float32)
    cp1 = np.broadcast_to(np.arange(1, 129, dtype=np.float32).reshape(128, 1), (128, 1))
    # pair selector for block (a, pr): column c has ones at row 96+j0 (even
    # rhs columns, channel j0 = a*4+2*pr) and row j1 (odd, j1 = j0+1)
    sel128 = np.zeros((128, 6 * NUM_CLASSES), dtype=np.float32)
    for a in range(3):
        for pr in range(2):
            blk = (a * 2 + pr) * NUM_CLASSES
            j0 = a * 4 + 2 * pr
            sel128[96 + j0, blk : blk + NUM_CLASSES] = 1.0
            sel128[j0 + 1, blk : blk + NUM_CLASSES] = 1.0
    blob = np.concatenate([offs_cols, hanch, ident, cp1, sel128], axis=1)
    return np.ascontiguousarray(blob.astype(np.float32))


def _host_prep(output, anchor, offset, stride):
    stride_f = float(stride)
    B = output.shape[0]
    x_all = np.ascontiguousarray(
        np.asarray(output, dtype=np.float32).reshape(B, HW_CELLS, NUM_ANCHOR * NUM_PRED)
    )
    consts = make_consts(anchor, offset, stride_f)
    return stride_f, x_all, consts


def kernel(output, anchor, offset, stride):
    from concourse.bass_utils import run_bass_kernel_spmd

    stride_f, x_all, consts = _host_prep(output, anchor, offset, stride)
    key = ("nc", stride_f, QBF16, RAMP, NVEC_COPY, IN_BUFS, O_BUFS)
    if key not in _CACHE:
        _CACHE[key] = _build(stride_f)
    nc = _CACHE[key]

    in_maps = [{"x": x_all[b], "consts": consts} for b in range(N_CORES)]
    res = run_bass_kernel_spmd(
        nc,
        in_maps,
        list(range(N_CORES)),
        tmpdir=os.environ.get("KERNEL_TRACE_DIR") or None,
    )
    global LAST_RESULT
    LAST_RESULT = res
    outs = [
        r["out"].reshape(NUM_CLASSES * HW_CELLS * NUM_ANCHOR, 6) for r in res.results
    ]
    return np.stack(outs, axis=0)


if __name__ == "__main__":
    rng = np.random.default_rng(0)
    out = rng.standard_normal((8, 80, 80, 255), dtype=np.float32)
    anchor = rng.uniform(10.0, 120.0, (1, 1, 3, 2)).astype(np.float32)
    gy, gx = np.meshgrid(np.arange(80, dtype=np.float32), np.arange(80, dtype=np.float32), indexing="ij")
    offset = np.stack([gx, gy], axis=-1).reshape(1, 80, 80, 1, 2)
    r = kernel(out, anchor, offset, 8)
    print(r.shape, r.dtype)
